# revision 23
# baseline (speedup 1.0000x reference)
"""Trainium2 Bass kernel for BoundaryLoss (data-parallel over batch).

Math (per batch sample b):
  mask  = boundary mask of target = (maxpool5x5(t) != minpool5x5(t)) with
          cv2-style clipped windows (OOB ignored).  Equals the reference's
          per-class dilate/erode union because a 5x5 window is non-uniform
          iff some class boundary passes through it.
  ce    = logsumexp_c(pred) - pred[t]
  wsum  = sum(mask * ce);  msum = sum(mask)
  per_sample = msum > 0 ? wsum/max(msum,1) : wsum/(H*W);  out = mean_b

Device algorithm (one sample per core), v3 (measured-rate tuned):
  - pred streams in via SWDGE *casting* DMA (fp32 HBM -> fp16 SBUF) in
    "layout B" [128, (4 rows, 512)] (partition p = rows 4p..4p+3), 8KB
    contiguous read runs, resident p16.  The 21 MB HBM read is the
    roofline.
  - S = sum_c exp(pred_c): exp on ACT emitting fp8e4 directly; summed
    over class PAIRS by DoubleRow identity-matmuls (2 k-tiles per pass,
    0.5 cyc/row) accumulating in PSUM.
  - picked = pred[t] per-pixel in PSUM ("P"): per class eq =
    tensor_scalar is_equal (DVE 4x, 0.69us) then o = eq*p16 tensor_tensor
    (DVE 2x, 1.18us) + 4 fp16 identity matmuls.  (stt / accum / select /
    copy_predicated all measured 1x on HW -- avoid them in the loop.)
  - boundary mask entirely in layout B, scheduled EARLY: two padded
    [128, 8, 520] tiles (rows 0..7 = global 4p-2..4p+5) built from two
    int32->fp16 casts + border memsets + partition-shift SBUF->SBUF DMAs
    on the sync queue right after the target cast (no mid-loop deps);
    horizontal 5-pools (8 rows), vertical 5-pools via strided row trees,
    neq -> maskb.  msum on ACT (activation accum).  No PSUM, no PE, no
    DRAM bounce for the mask.
  - finals: ln(S) on ACT (fp16 out), j2 = sum(mask*P) emitted before
    j1 = sum(mask*lse) so j2 overlaps the Ln; partition-reduce via
    ones-matmuls; DMA out [1,32].  Host combines per-core outputs.
"""

import numpy as np

B = 8
C = 21
H = 512
W = 512
N_CORES = 8
G4 = 4          # row groups (H = 128 * G4)
PW = 520        # padded width for horizontal pooling; data cols [2, 514)
CHUNKS = [2, 2, 4, 4, 4, 4, 1]  # class chunking (sum = C)

_CACHE = {}


def _patch_act_tables(bacc_mod, mybir, arch):
    """Steer the act-table-load pass to the set containing BOTH exp and ln
    (one table load total instead of an exp-table load up front plus an
    ln-table reload in the serial tail): present every other set as empty
    so the greedy selection can only pick the combined one."""
    try:
        from concourse.hw_specs import get_activation_tables

        orig = get_activation_tables(arch)
        Fn = mybir.ActivationFunctionType
        need = {Fn.Exp, Fn.Ln, Fn.Copy}
        combined = next(name for name, fns in orig.items()
                        if need.issubset(fns))
        tables = {name: (fns if name == combined else set())
                  for name, fns in orig.items()}
        bacc_mod.get_activation_tables = lambda _arch: tables
    except Exception:
        pass


def _build_nc():
    from contextlib import ExitStack

    import concourse.bacc as bacc
    import concourse.tile as tile
    from concourse import mybir
    from concourse.masks import make_identity

    dt = mybir.dt
    Alu = mybir.AluOpType
    Act = mybir.ActivationFunctionType

    nc = bacc.Bacc("TRN2", target_bir_lowering=False, debug=False,
                   num_devices=N_CORES)
    _patch_act_tables(bacc, mybir, nc.m.arch)

    pred = nc.dram_tensor("pred", [C, H, W], dt.float32, kind="ExternalInput")
    target = nc.dram_tensor("target", [H, W], dt.int32, kind="ExternalInput")
    out = nc.dram_tensor("out", [1, 32], dt.float32, kind="ExternalOutput")

    with tile.TileContext(nc) as tc, ExitStack() as ctx:
        consts = ctx.enter_context(tc.tile_pool(name="consts", bufs=1))
        keep = ctx.enter_context(tc.tile_pool(name="keep", bufs=1))
        mp = ctx.enter_context(tc.tile_pool(name="maskpool", bufs=1))
        ms = ctx.enter_context(tc.tile_pool(name="maskscratch", bufs=1))
        epool = ctx.enter_context(tc.tile_pool(name="ep", bufs=2))
        opool = ctx.enter_context(tc.tile_pool(name="op", bufs=2))
        fin = ctx.enter_context(tc.tile_pool(name="fin", bufs=1))
        dramp = ctx.enter_context(tc.tile_pool(name="dram", bufs=1,
                                               space="DRAM"))
        sps = ctx.enter_context(tc.tile_pool(name="spsum", bufs=1,
                                             space="PSUM"))
        pps = ctx.enter_context(tc.tile_pool(name="ppsum", bufs=1,
                                             space="PSUM"))

        ident16 = consts.tile([128, 128], dt.float16)
        make_identity(nc, ident16)
        ident8x2 = consts.tile([128, 2, 128], dt.float8e4)
        make_identity(nc, ident8x2[:, 0, :])
        make_identity(nc, ident8x2[:, 1, :])
        ones = consts.tile([128, 1], dt.float32)
        nc.gpsimd.memset(ones, 1.0)
        warm = consts.tile([128, 512], dt.float16)
        nc.gpsimd.memset(warm, 0.0)
        st_w1 = consts.tile([128, 1], dt.float32)   # sum mask*lse
        st_l2 = consts.tile([128, 1], dt.float32)   # sum mask*picked
        st_m = consts.tile([128, 1], dt.float32)    # sum mask

        # ---------------- resident tensors (layout B) ----------------
        p16 = keep.tile([128, C, G4, W], dt.float16)   # pred, fp16

        # ---------------- early loads ----------------
        # Target with an 8-row halo per partition (rows 4p-2 .. 4p+5).
        # Partial-partition DMAs measure ~100x slow on HW, so every SBUF
        # load here is full-range [0:128]: the halo rows come from a
        # DRAM staging buffer padded by edge replication (replicate ==
        # clip for min/max pooling), built with row-granular DRAM->DRAM
        # copies.
        tpad = dramp.tile([4 + H, W], dt.int32)
        tap = target.ap()
        nc.sync.dma_start(out=tpad[2:2 + H, :], in_=tap)
        nc.sync.dma_start(out=tpad[0:1, :], in_=tap[0:1])
        nc.sync.dma_start(out=tpad[1:2, :], in_=tap[0:1])
        nc.sync.dma_start(out=tpad[2 + H:3 + H, :], in_=tap[H - 1:H])
        nc.sync.dma_start(out=tpad[3 + H:4 + H, :], in_=tap[H - 1:H])
        t32e = mp.tile([128, 8, W], dt.int32, tag="t32e")
        # tile rows 0..3 = global 4p-2..4p+1 = tpad rows 4p..4p+3;
        # tile rows 4..7 = global 4p+2..4p+5 = tpad rows 4p+4..4p+7.
        # Both are full-range [0:128] stride-4 reads of the padded buffer.
        nc.sync.dma_start(
            out=t32e[:, 0:4, :],
            in_=tpad[0:512].rearrange("(p r) w -> p r w", p=128))
        nc.sync.dma_start(
            out=t32e[:, 4:8, :],
            in_=tpad[4:4 + 512].rearrange("(p r) w -> p r w", p=128))

        # pred chunk DMAs: issue ALL up front on the SWDGE queue so the
        # descriptor stream never bubbles (p16 is resident, no pool deps).
        starts = []
        c0 = 0
        for n in CHUNKS:
            starts.append((c0, n))
            nc.gpsimd.dma_start(
                out=p16[:, c0:c0 + n, :, :],
                in_=pred.ap()[c0:c0 + n].rearrange(
                    "c (p r) w -> p c r w", p=128))
            c0 += n

        # ---------------- mask setup (all layout B, scheduled early) ----
        # padded 8-row tiles: row i = global row 4p + i - 2, cols [2, 514)
        xmax8 = mp.tile([128, 8, PW], dt.float16, tag="xmax8")
        xmin8 = mp.tile([128, 8, PW], dt.float16, tag="xmin8")
        for t, v in ((xmax8, -1.0), (xmin8, 99.0)):
            nc.gpsimd.memset(t[:, :, 0:2], v)            # w borders
            nc.gpsimd.memset(t[:, :, 2 + W:PW], v)
        # target -> fp16 into both padded tiles (all 8 halo rows; eq reads
        # the core rows of xmax8).  Row borders are already edge-replicated
        # via tpad, which is max/min-neutral -- no fixups needed.
        tb = xmax8[:, 2:6, 2:2 + W]
        nc.vector.tensor_copy(out=xmax8[:, 0:8, 2:2 + W], in_=t32e)
        nc.vector.tensor_copy(out=xmin8[:, 0:8, 2:2 + W], in_=t32e)

        # PE warmup into the future P bank (discarded by c==0's start=True)
        s_ps = sps.tile([128, G4, W], dt.float32, tag="s")
        p_ps = pps.tile([128, G4, W], dt.float32, tag="p")
        for _ in range(10):
            nc.tensor.matmul(p_ps[:, 0, :], ident16, warm, start=True,
                             stop=True)

        hx8 = mp.tile([128, 8, W], dt.float16, tag="h8")
        hn8 = mp.tile([128, 8, W], dt.float16, tag="h8")  # reuse slot
        vx = mp.tile([128, G4, W], dt.float16, tag="vx")
        vn = mp.tile([128, G4, W], dt.float16, tag="vn")
        maskb = keep.tile([128, G4, W], dt.float16)
        junk = mp.tile([128, G4, W], dt.float16, tag="junk")

        def hpool(src, op, dst):
            # horizontal 5-pool over all 8 rows
            m2 = ms.tile([128, 8, PW], dt.float16, tag="m2")
            m4 = ms.tile([128, 8, PW], dt.float16, tag="m4")
            nc.vector.tensor_tensor(
                out=m2[:, :, 0:PW - 1],
                in0=src[:, :, 0:PW - 1], in1=src[:, :, 1:PW], op=op)
            nc.vector.tensor_tensor(
                out=m4[:, :, 0:PW - 3],
                in0=m2[:, :, 0:PW - 3], in1=m2[:, :, 2:PW - 1], op=op)
            nc.vector.tensor_tensor(
                out=dst, in0=m4[:, :, 0:W], in1=src[:, :, 4:4 + W], op=op)

        def vpool(ext, op, dst):
            # vertical 5-pool: out row r needs ext rows r..r+4
            # (scratch reuses the hpool buffers: same tag + shape, sliced)
            m2f = ms.tile([128, 8, PW], dt.float16, tag="m2")
            m4f = ms.tile([128, 8, PW], dt.float16, tag="m4")
            m2 = m2f[:, 0:7, 0:W]
            m4 = m4f[:, 0:5, 0:W]
            nc.vector.tensor_tensor(
                out=m2, in0=ext[:, 0:7, :], in1=ext[:, 1:8, :], op=op)
            nc.vector.tensor_tensor(
                out=m4, in0=m2[:, 0:5, :], in1=m2[:, 2:7, :], op=op)
            nc.vector.tensor_tensor(
                out=dst, in0=m4[:, 0:4, :], in1=ext[:, 4:8, :], op=op)

        def st_neq():
            nc.vector.tensor_tensor(out=maskb, in0=vx, in1=vn,
                                    op=Alu.not_equal)

        def st_msum():
            # own scratch output (m2 slot, dead by now): sharing `junk`
            # with j1/j2 creates a cross-engine WAW chain into the tail
            jmf = ms.tile([128, 8, PW], dt.float16, tag="m2")
            nc.scalar.activation(out=jmf[:, 0:4, 0:W], in_=maskb,
                                 func=Act.Copy, accum_out=st_m)

        stages = [
            lambda: hpool(xmax8, Alu.max, hx8),
            lambda: vpool(hx8, Alu.max, vx),
            lambda: hpool(xmin8, Alu.min, hn8),
            lambda: vpool(hn8, Alu.min, vn),
            st_neq,
            st_msum,
        ]

        # ---------------- class loop, stages interleaved ----------------
        DR = mybir.MatmulPerfMode.DoubleRow
        for k, (c0, nct) in enumerate(starts):
            if k < len(stages):
                stages[k]()
            e8 = epool.tile([128, 4, G4, W], dt.float8e4, tag="e")
            nc.scalar.activation(out=e8[:, 0:nct, :, :],
                                 in_=p16[:, c0:c0 + nct, :, :], func=Act.Exp)
            # S accumulation: DoubleRow over class pairs (fp8, 2 k-tiles)
            for a in range(0, nct - 1, 2):
                for j in range(G4):
                    nc.tensor.matmul(
                        s_ps[:, j, :], ident8x2, e8[:, a:a + 2, j, :],
                        start=(c0 + a == 0), stop=False, perf_mode=DR)
            if nct % 2:  # solo class (the last chunk)
                for j in range(G4):
                    nc.tensor.matmul(
                        s_ps[:, j, :], ident8x2[:, 0, :],
                        e8[:, nct - 1, j, :],
                        start=(c0 + nct - 1 == 0), stop=(c0 + nct == C))
            # picked accumulation: eq (4x) + product (2x) + fp16 matmuls
            for i in range(nct):
                c = c0 + i
                eq_t = opool.tile([128, G4, W], dt.float16, tag="q")
                nc.vector.tensor_scalar(
                    out=eq_t, in0=tb, scalar1=float(c), scalar2=None,
                    op0=Alu.is_equal)
                o_t = opool.tile([128, G4, W], dt.float16, tag="o")
                nc.vector.tensor_tensor(
                    out=o_t, in0=eq_t, in1=p16[:, c, :, :], op=Alu.mult)
                for j in range(G4):
                    nc.tensor.matmul(
                        p_ps[:, j, :], ident16, o_t[:, j, :],
                        start=(c == 0), stop=(c == C - 1))
        for k in range(len(starts), len(stages)):
            stages[k]()

        # ---------------- finals ----------------
        # j2 first on DVE: overlaps the Ln on ACT
        nc.vector.scalar_tensor_tensor(
            out=junk, in0=p_ps, scalar=0.0, in1=maskb,
            op0=Alu.add, op1=Alu.mult, accum_out=st_l2)
        l1 = fin.tile([128, G4, W], dt.float16)
        nc.scalar.activation(out=l1, in_=s_ps, func=Act.Ln)
        nc.vector.scalar_tensor_tensor(
            out=junk, in0=l1, scalar=0.0, in1=maskb,
            op0=Alu.add, op1=Alu.mult, accum_out=st_w1)

        # partition reductions — reuse the S bank (fully consumed by l1)
        red = s_ps[0:1, 0, 0:32]
        nc.tensor.matmul(red[:, 0:1], ones, st_w1, start=True, stop=True)
        nc.tensor.matmul(red[:, 1:2], ones, st_l2, start=True, stop=True)
        nc.tensor.matmul(red[:, 2:3], ones, st_m, start=True, stop=True)
        outsb = consts.tile([1, 32], dt.float32)
        nc.vector.memset(outsb, 0.0)
        nc.vector.tensor_copy(out=outsb[:, 0:3], in_=red[:, 0:3])
        nc.sync.dma_start(out=out.ap(), in_=outsb)

    nc.compile()
    return nc


def get_nc():
    if "nc" not in _CACHE:
        _CACHE["nc"] = _build_nc()
    return _CACHE["nc"]


def _combine(outs):
    """outs: list of per-core [1,32] float32 -> scalar loss."""
    per_sample = []
    for o in outs:
        w1, l2, msum = float(o[0, 0]), float(o[0, 1]), float(o[0, 2])
        wsum = w1 - l2
        if msum > 0:
            per_sample.append(wsum / max(msum, 1.0))
        else:
            per_sample.append(wsum / float(H * W))
    return np.float32(np.mean(per_sample))


def kernel(pred, target):
    from concourse.bass_utils import run_bass_kernel_spmd

    pred = np.ascontiguousarray(pred, dtype=np.float32)
    target = np.ascontiguousarray(target, dtype=np.int32)
    assert pred.shape == (B, C, H, W) and target.shape == (B, H, W)

    nc = get_nc()
    in_maps = [{"pred": pred[b], "target": target[b]} for b in range(B)]
    res = run_bass_kernel_spmd(nc, in_maps, core_ids=list(range(N_CORES)))
    outs = [res.results[b]["out"] for b in range(B)]
    return np.asarray(_combine(outs), dtype=np.float32)


# revision 25
# speedup vs baseline: 1.1487x; 1.1487x over previous
"""Trainium2 Bass kernel for BoundaryLoss (data-parallel over batch).

Math (per batch sample b):
  mask  = boundary mask of target = (maxpool5x5(t) != minpool5x5(t)) with
          cv2-style clipped windows (OOB ignored).  Equals the reference's
          per-class dilate/erode union because a 5x5 window is non-uniform
          iff some class boundary passes through it.
  ce    = logsumexp_c(pred) - pred[t]
  wsum  = sum(mask * ce);  msum = sum(mask)
  per_sample = msum > 0 ? wsum/max(msum,1) : wsum/(H*W);  out = mean_b

Device algorithm (one sample per core):
  - pred streams in "layout B" [128, (4 rows, 512)] (partition p = rows
    4p..4p+3) giving 8KB-contiguous DMA runs (~400+ GB/s measured) — the
    21 MB pred stream is the roofline for this kernel.
  - S = sum_c exp(pred_c): exp on ACT (fp16 out), summed over classes by
    identity-matmul PSUM accumulation on TensorE.
  - picked = pred[t], mask-weighted, is gathered two ways:
      early classes (before the mask is ready): eq=(t==c) on DVE 4x, then
        copy_predicated G[t==c] = e_c into SBUF; finals add
        sum(mask*ln(G)) (G init to 1 so untouched pixels contribute 0).
      late classes (K0 < C only; currently disabled, K0=C — holding raw
        pred tiles for the mask-gated fused-stt path stalled the DMA
        stream more than the DVE savings were worth).
  - boundary mask concurrently in "layout A" [128, (4, 512)] (partition =
    row g*128+p): horizontal 5-max/min via 3 shifted tensor_tensor ops,
    PE-transpose 128x128 blocks (PSUM), vertical pools in transposed
    space, compare, PE-transpose back, bounce through DRAM into layout B.
    Emission is interleaved between class chunks so every engine's
    (statically ordered) instruction stream stays dependency-ready.
  - finals: sum(mask*ln(S)) and sum(mask*ln(G)) via stt accum; msum via
    ACT accum; partition-reduce via ones-matmuls; DMA out [1,32].
Host combines the per-core outputs.
"""

import numpy as np

B = 8
C = 21
H = 512
W = 512
N_CORES = 8
CHUNK = 2  # pred planes per DMA
K0 = 21  # classes [0, K0) use copy_predicated; [K0, C) use masked stt accum
PW = 520  # padded width of pooling buffers; data cols [2, 514)
G4 = 4  # row groups (H = G4 * 128)

_CACHE = {}


def _patch_act_tables(bacc_mod, mybir, arch):
    """Steer the act-table-load pass to the set containing BOTH exp and ln
    (one table load total instead of an exp-table load up front plus an
    ln-table reload in the serial tail): present every other set as empty
    so the greedy selection can only pick the combined one.  Set ids and
    the act_info.json walrus reads stay untouched."""
    try:
        from concourse.hw_specs import get_activation_tables

        orig = get_activation_tables(arch)
        Fn = mybir.ActivationFunctionType
        need = {Fn.Exp, Fn.Ln, Fn.Copy}
        combined = next(name for name, fns in orig.items()
                        if need.issubset(fns))
        tables = {name: (fns if name == combined else set())
                  for name, fns in orig.items()}
        bacc_mod.get_activation_tables = lambda _arch: tables
    except Exception:
        pass


def _build_nc():
    from contextlib import ExitStack

    import concourse.bacc as bacc
    import concourse.tile as tile
    from concourse import mybir
    from concourse.masks import make_identity

    dt = mybir.dt
    Alu = mybir.AluOpType
    Act = mybir.ActivationFunctionType

    nc = bacc.Bacc("TRN2", target_bir_lowering=False, debug=False,
                   num_devices=N_CORES)
    _patch_act_tables(bacc, mybir, nc.m.arch)

    pred = nc.dram_tensor("pred", [C, H, W], dt.float32, kind="ExternalInput")
    target = nc.dram_tensor("target", [H, W], dt.int32, kind="ExternalInput")
    out = nc.dram_tensor("out", [1, 32], dt.float32, kind="ExternalOutput")

    with tile.TileContext(nc) as tc, ExitStack() as ctx:
        consts = ctx.enter_context(tc.tile_pool(name="consts", bufs=1))
        keep = ctx.enter_context(tc.tile_pool(name="keep", bufs=1))
        mp = ctx.enter_context(tc.tile_pool(name="maskpool", bufs=1))
        ms = ctx.enter_context(tc.tile_pool(name="maskscratch", bufs=1))
        ppool = ctx.enter_context(tc.tile_pool(name="pp", bufs=3))
        epool = ctx.enter_context(tc.tile_pool(name="ep", bufs=3))
        qpool = ctx.enter_context(tc.tile_pool(name="qp", bufs=4))
        jpool = ctx.enter_context(tc.tile_pool(name="jp", bufs=2))
        opool = ctx.enter_context(tc.tile_pool(name="op", bufs=4))
        fin = ctx.enter_context(tc.tile_pool(name="fin", bufs=1))
        dramp = ctx.enter_context(tc.tile_pool(name="dram", bufs=1,
                                               space="DRAM"))
        mps = ctx.enter_context(tc.tile_pool(name="mpsum", bufs=1,
                                             space="PSUM"))
        sgp = ctx.enter_context(tc.tile_pool(name="sgpsum", bufs=1,
                                             space="PSUM"))

        ident = consts.tile([128, 128], dt.float16)
        make_identity(nc, ident)
        ones = consts.tile([128, 1], dt.float32)
        nc.gpsimd.memset(ones, 1.0)
        warm = consts.tile([128, 512], dt.float16)
        nc.gpsimd.memset(warm, 0.0)
        st_w1 = consts.tile([128, 1], dt.float32)
        st_l2 = consts.tile([128, 1], dt.float32)
        st_m = consts.tile([128, 1], dt.float32)
        st2 = consts.tile([128, 16], dt.float32)  # auxiliary l2 accums
        nc.vector.memset(st2, 0.0)

        # layout-B tensors
        tb = keep.tile([128, G4, W], dt.float16)      # target as fp16
        maskb = keep.tile([128, G4, W], dt.float16)   # mask (from bounce)
        g_sb = keep.tile([128, 1, W], dt.float16)     # r=3 gather | 1.0
        mask_dram = dramp.tile([H, W], dt.float16)

        # ---------------- early loads ----------------
        # Target loads ride the (otherwise idle) SWDGE queue so they don't
        # queue ahead of pred chunk 0 on the sync queue: the 2MB of
        # 2KB-descriptor target DMAs were delaying exp0 by ~15us.
        t32 = mp.tile([128, G4, W], dt.int32)
        nc.gpsimd.dma_start(
            out=t32, in_=target.ap().rearrange("(g p) w -> p g w", p=128))
        t32b = mp.tile([128, G4, W], dt.int32, tag="t32b")
        nc.gpsimd.dma_start(
            out=t32b, in_=target.ap().rearrange("(p r) w -> p r w", p=128))
        nc.vector.tensor_copy(out=tb, in_=t32b)
        nc.gpsimd.memset(g_sb, 1.0)

        # PE warmup into the future S bank (discarded by c==0's start=True)
        s_ps = sgp.tile([128, G4, W], dt.float32, tag="s")
        g_ps = sgp.tile([128, 3, W], dt.float32, tag="g")
        for _ in range(10):
            nc.tensor.matmul(s_ps[:, 0, :], ident, warm, start=True,
                             stop=True)

        # ---------------- mask pipeline stages (layout A) ----------------
        xmax = mp.tile([128, G4, PW], dt.float16, tag="xmax")
        xmin = mp.tile([128, G4, PW], dt.float16, tag="xmin")
        xt = mp.tile([128, G4, PW], dt.float16, tag="xt")
        xnt = mp.tile([128, G4, PW], dt.float16, tag="xnt")
        for t in (xmax, xt):
            nc.gpsimd.memset(t[:, :, 0:2], -1.0)
            nc.gpsimd.memset(t[:, :, 2 + W:PW], -1.0)
        for t in (xmin, xnt):
            nc.gpsimd.memset(t[:, :, 0:2], 99.0)
            nc.gpsimd.memset(t[:, :, 2 + W:PW], 99.0)
        hx = mp.tile([128, G4, W], dt.float16, tag="hx")
        hn = mp.tile([128, G4, W], dt.float16, tag="hn")
        vx = mp.tile([128, G4, W], dt.float16, tag="hx")   # reuse slot
        vn = mp.tile([128, G4, W], dt.float16, tag="hn")   # reuse slot
        maskt = mp.tile([128, G4, W], dt.float16, tag="maskt")
        mask_a = mp.tile([128, G4, W], dt.float16, tag="maska")

        def pool5(src, op, dst):
            m2 = ms.tile([128, G4, PW], dt.float16, tag="m2")
            m4 = ms.tile([128, G4, PW], dt.float16, tag="m4")
            nc.vector.tensor_tensor(
                out=m2[:, :, 0:PW - 1],
                in0=src[:, :, 0:PW - 1], in1=src[:, :, 1:PW], op=op)
            nc.vector.tensor_tensor(
                out=m4[:, :, 0:PW - 3],
                in0=m2[:, :, 0:PW - 3], in1=m2[:, :, 2:PW - 1], op=op)
            nc.vector.tensor_tensor(
                out=dst, in0=m4[:, :, 0:W], in1=src[:, :, 4:4 + W], op=op)

        def tpose_in(src, dst):
            for q in range(4):
                tq = mps.tile([128, 512], dt.float16, tag="tq")
                for g in range(4):
                    nc.tensor.transpose(
                        tq[:, g * 128:(g + 1) * 128],
                        src[:, g, q * 128:(q + 1) * 128], ident)
                nc.scalar.copy(out=dst[:, q, 2:2 + W], in_=tq)

        def st_casts():
            nc.vector.tensor_copy(out=xmax[:, :, 2:2 + W], in_=t32)
            nc.vector.tensor_copy(out=xmin[:, :, 2:2 + W], in_=t32)

        def st_neq():
            nc.vector.tensor_tensor(out=maskt, in0=vx, in1=vn,
                                    op=Alu.not_equal)
            junk_m = ms.tile([128, G4, W], dt.float16, tag="junkm")
            nc.scalar.activation(out=junk_m, in_=maskt, func=Act.Copy,
                                 accum_out=st_m)

        def st_back():
            for g in range(4):
                tg = mps.tile([128, 512], dt.float16, tag="tq")
                for q in range(4):
                    nc.tensor.transpose(
                        tg[:, q * 128:(q + 1) * 128],
                        maskt[:, q, g * 128:(g + 1) * 128], ident)
                nc.scalar.copy(out=mask_a[:, g, :], in_=tg)

        def st_bounce():
            nc.gpsimd.dma_start(
                out=mask_dram[:].rearrange("(g p) w -> p g w", p=128),
                in_=mask_a)
            nc.gpsimd.dma_start(
                out=maskb,
                in_=mask_dram[:].rearrange("(p r) w -> p r w", p=128))

        def st_tt2():
            # tt2b = (t+1) * mask, in layout B
            nc.vector.scalar_tensor_tensor(
                out=tt2b, in0=tb, scalar=1.0, in1=maskb,
                op0=Alu.add, op1=Alu.mult)

        stages = [
            st_casts,
            lambda: pool5(xmax, Alu.max, hx),
            lambda: pool5(xmin, Alu.min, hn),
            lambda: tpose_in(hx, xt),
            lambda: tpose_in(hn, xnt),
            lambda: pool5(xt, Alu.max, vx),
            lambda: pool5(xnt, Alu.min, vn),
            st_neq,
            st_back,
            st_bounce,
        ]

        # ---------------- class loop (layout B), stages interleaved -------
        starts = list(range(0, C, CHUNK))
        for k, c0 in enumerate(starts):
            if k < len(stages):
                stages[k]()
            nct = min(CHUNK, C - c0)
            p_t = ppool.tile([128, nct, G4, W], dt.float32, tag="p")
            nc.sync.dma_start(
                out=p_t,
                in_=pred.ap()[c0:c0 + nct].rearrange(
                    "c (p r) w -> p c r w", p=128))
            e_t = epool.tile([128, nct, G4, W], dt.float16, tag="e")
            nc.scalar.activation(out=e_t, in_=p_t, func=Act.Exp)
            for i in range(nct):
                c = c0 + i
                eq_t = qpool.tile([128, G4, W], dt.uint16, tag="q")
                nc.vector.tensor_scalar(
                    out=eq_t, in0=tb, scalar1=float(c), scalar2=None,
                    op0=Alu.is_equal)
                # rows 0..2: gather via 2x multiply + identity matmul
                o_t = opool.tile([128, 3, W], dt.float16, tag="o")
                nc.vector.tensor_tensor(
                    out=o_t, in0=eq_t[:, 0:3, :], in1=e_t[:, i, 0:3, :],
                    op=Alu.mult)
                # row 3: gather via predicated overwrite (1x but quarter-FD)
                nc.vector.copy_predicated(out=g_sb[:, 0, :],
                                          mask=eq_t[:, 3, :],
                                          data=e_t[:, i, 3, :])
                for j in range(4):
                    nc.tensor.matmul(
                        s_ps[:, j, :], ident, e_t[:, i, j, :],
                        start=(c == 0), stop=(c == C - 1))
                for j in range(3):
                    nc.tensor.matmul(
                        g_ps[:, j, :], ident, o_t[:, j, :],
                        start=(c == 0), stop=(c == C - 1))
        for k in range(len(starts), len(stages)):
            stages[k]()

        # ---------------- finals ----------------
        l1 = fin.tile([128, G4, W], dt.float32)
        nc.scalar.activation(out=l1, in_=s_ps, func=Act.Ln)
        lg3 = fin.tile([128, 3, W], dt.float32)
        nc.scalar.activation(out=lg3, in_=g_ps, func=Act.Ln)
        lg4 = fin.tile([128, 1, W], dt.float32)
        nc.scalar.activation(out=lg4, in_=g_sb, func=Act.Ln)

        j1 = jpool.tile([128, G4, W], dt.float32, tag="junk")
        nc.vector.scalar_tensor_tensor(
            out=j1, in0=l1, scalar=0.0, in1=maskb,
            op0=Alu.add, op1=Alu.mult, accum_out=st_w1)
        j2 = jpool.tile([128, G4, W], dt.float32, tag="junk")
        nc.vector.scalar_tensor_tensor(
            out=j2[:, 0:3, :], in0=lg3, scalar=0.0, in1=maskb[:, 0:3, :],
            op0=Alu.add, op1=Alu.mult, accum_out=st_l2)
        nc.vector.scalar_tensor_tensor(
            out=j2[:, 3:4, :], in0=lg4, scalar=0.0, in1=maskb[:, 3:4, :],
            op0=Alu.add, op1=Alu.mult, accum_out=st2[:, 0:1])

        # partition reductions — reuse the S bank (fully consumed by l1)
        red = s_ps[0:1, 0, 0:32]
        nc.tensor.matmul(red[:, 0:1], ones, st_w1, start=True, stop=True)
        nc.tensor.matmul(red[:, 1:2], ones, st_l2, start=True, stop=True)
        nc.tensor.matmul(red[:, 2:3], ones, st_m, start=True, stop=True)
        nc.tensor.matmul(red[:, 8:24], ones, st2, start=True, stop=True)
        outsb = consts.tile([1, 32], dt.float32)
        nc.vector.memset(outsb, 0.0)
        nc.vector.tensor_copy(out=outsb[:, 0:3], in_=red[:, 0:3])
        nc.vector.tensor_copy(out=outsb[:, 8:24], in_=red[:, 8:24])
        nc.sync.dma_start(out=out.ap(), in_=outsb)

    nc.compile()
    return nc


def get_nc():
    if "nc" not in _CACHE:
        _CACHE["nc"] = _build_nc()
    return _CACHE["nc"]


def _combine(outs):
    """outs: list of per-core [1,32] float32 -> scalar loss."""
    per_sample = []
    for o in outs:
        w1, l2, msum = float(o[0, 0]), float(o[0, 1]), float(o[0, 2])
        l2 += float(o[0, 8:24].sum())  # auxiliary l2 partial sums
        wsum = w1 - l2
        if msum > 0:
            per_sample.append(wsum / max(msum, 1.0))
        else:
            per_sample.append(wsum / float(H * W))
    return np.float32(np.mean(per_sample))


def kernel(pred, target):
    from concourse.bass_utils import run_bass_kernel_spmd

    pred = np.ascontiguousarray(pred, dtype=np.float32)
    target = np.ascontiguousarray(target, dtype=np.int32)
    assert pred.shape == (B, C, H, W) and target.shape == (B, H, W)

    nc = get_nc()
    in_maps = [{"pred": pred[b], "target": target[b]} for b in range(B)]
    res = run_bass_kernel_spmd(nc, in_maps, core_ids=list(range(N_CORES)))
    outs = [res.results[b]["out"] for b in range(B)]
    return np.asarray(_combine(outs), dtype=np.float32)
